# revision 24
# baseline (speedup 1.0000x reference)
"""Causal self-attention Trainium2 Bass kernel (v2, bf16 pipeline).

Problem: nn_CausalSelfAttention (B=2, L=2048, D=1024, H=16 heads, Khd=64).

Sharding (8 cores): data-parallel over B (2 way) x tensor-parallel over
heads (4 way, 4 heads/core).  Each core computes
  qkv_local = x_b @ W_attn_local.T          (c_attn column-sharded)
  attn_local = causal_attention(q,k,v)      (4 heads)
  y_partial  = attn_local @ W_proj_local.T  (c_proj row-sharded)
and the host sums the 4 partials per batch (the row-parallel unshard).

v2 changes over the fp32r baseline:
  - whole pipeline in bf16 (fp32 PSUM accumulation): halves DMA bytes,
    removes the fp32r 4x penalty on <256-wide matmuls, halves SBUF.
  - inputs host-packed into k-interleaved layouts so each big tensor
    loads with ONE wide DMA (9 input DMAs total vs ~60): kills the
    ~630ns/DMA HWDGE serialization that stalled the start.
  - v_aug ones-column via memset instead of 16 tiny DMAs.
  - emission starts with emit_v on the first xT column block so PE
    ramps while the rest of xT streams in.
  - q/k PSUM->SBUF copies split DVE/ACT; tail proj copies on ACT.
  - y written bf16 via [128,1024] staging tiles (16 output DMAs).
"""

import math

import numpy as np

B, L, D, H = 2, 2048, 1024, 16
KHD = D // H  # 64 head dim
NCORES = 8
HPC = 4  # heads per core
FQK = 2 * HPC * KHD  # 512 q+k local features
FV = HPC * KHD  # 256 v local features
FQKV = FQK + FV  # 768
DK = D // 128  # 8 contraction chunks
LC = L // 128  # 16 row chunks
NJ = L // 512  # 4 qrow blocks
NXB = 4  # xT column blocks of 512
SCALE = 1.0 / math.sqrt(KHD)

_CACHE = {}


def _build(has_bqk: bool, has_bv: bool, has_bp: bool, reps: int = 1):
    import concourse.bass as bass
    import concourse.mybir as mybir
    import concourse.tile as tile
    from concourse import bacc

    f32 = mybir.dt.float32
    bf16 = mybir.dt.bfloat16

    nc = bacc.Bacc(None, target_bir_lowering=False)
    # host-packed k-interleaved layouts: one wide DMA per tensor
    xTi_d = nc.declare_dram_parameter("xTi", [128, NXB, DK, 512], bf16, isOutput=False)
    wqkvTi_d = nc.declare_dram_parameter("wqkvTi", [128, DK, FQKV], bf16, isOutput=False)
    wpTi_d = nc.declare_dram_parameter("wpTi", [128, 2, D], bf16, isOutput=False)
    tri_d = nc.declare_dram_parameter("tri", [128, 128], bf16, isOutput=False)
    if has_bqk:
        bqk_d = nc.declare_dram_parameter("bqk", [128, FQK // 128], f32, isOutput=False)
    if has_bv or has_bp:
        onesrow_d = nc.declare_dram_parameter("onesrow", [1, 128], bf16, isOutput=False)
    if has_bv:
        bv_d = nc.declare_dram_parameter("bv", [1, FV], bf16, isOutput=False)
    if has_bp:
        bp_d = nc.declare_dram_parameter("bp", [1, D], bf16, isOutput=False)
    y_d = nc.declare_dram_parameter("y", [L, D], bf16, isOutput=True)

    with nc.allow_low_precision(reason="bf16 matmul pipeline, fp32 accum"), tile.TileContext(nc) as tc:
        with (
            tc.tile_pool(name="persist", bufs=1) as persist,
            tc.tile_pool(name="work", bufs=3) as work,
            tc.tile_pool(name="small", bufs=2) as small,
            tc.tile_pool(name="ps_sc", bufs=2, space="PSUM") as ps_sc,
            tc.tile_pool(name="ps_out", bufs=2, space="PSUM") as ps_out,
            tc.tile_pool(name="ps_y", bufs=2, space="PSUM") as ps_y,
        ):
            for _rep in range(reps):
                # ---- persistent SBUF tensors ----
                xTi_sbs = [
                    persist.tile([128, DK, 512], bf16, name=f"xTi{c}", tag=f"xTi{c}")
                    for c in range(NXB)
                ]
                wqkvTi_sb = persist.tile([128, DK, FQKV], bf16, tag="wqkvTi")
                qT_p = [persist.tile([128, L], bf16, name=f"qT{p}", tag=f"qT{p}") for p in range(2)]
                kT_p = [persist.tile([128, L], bf16, name=f"kT{p}", tag=f"kT{p}") for p in range(2)]
                v_augs = [
                    persist.tile([128, HPC, KHD + 1], bf16, name=f"vaug{lc}", tag=f"vaug{lc}")
                    for lc in range(LC)
                ]
                attnT_js = [
                    persist.tile([128, 2, 512], bf16, name=f"attnT{j}", tag=f"attnT{j}")
                    for j in range(NJ)
                ]
                wpTi_sb = persist.tile([128, 2, D], bf16, tag="wpTi")
                tri_sb = persist.tile([128, 128], bf16)

                # ---- input DMAs: few, wide, priority-ordered; none on ACT
                # (it needs every cycle for exp).  First emit_v(0..3) needs
                # only xTi[0] cols 0:128-per-k and wqkvTi v-cols: load those
                # first so PE starts ~3us in.
                nc.sync.dma_start(
                    out=xTi_sbs[0][:, :, 0:128], in_=xTi_d[:, 0, :, 0:128]
                )
                nc.gpsimd.dma_start(
                    out=wqkvTi_sb[:, :, FQK:FQKV], in_=wqkvTi_d[:, :, FQK:FQKV]
                )
                nc.sync.dma_start(
                    out=xTi_sbs[0][:, :, 128:512], in_=xTi_d[:, 0, :, 128:512]
                )
                nc.gpsimd.dma_start(
                    out=wqkvTi_sb[:, :, 0:FQK], in_=wqkvTi_d[:, :, 0:FQK]
                )
                for c in range(1, NXB):
                    nc.sync.dma_start(out=xTi_sbs[c], in_=xTi_d[:, c])
                nc.gpsimd.dma_start(out=wpTi_sb, in_=wpTi_d[:])
                nc.gpsimd.dma_start(out=tri_sb, in_=tri_d[:])
                # ones column of v_aug (softmax denominator trick)
                for lc in range(LC):
                    nc.gpsimd.memset(v_augs[lc][:, :, KHD], 1.0)
                if has_bqk:
                    bqk_sb = persist.tile([128, FQK // 128], f32)
                    nc.sync.dma_start(out=bqk_sb, in_=bqk_d[:])
                if has_bv or has_bp:
                    ones_row = persist.tile([1, 128], bf16)
                    nc.sync.dma_start(out=ones_row, in_=onesrow_d[:])
                if has_bv:
                    bv_sb = persist.tile([1, FV], bf16)
                    nc.sync.dma_start(out=bv_sb, in_=bv_d[:])
                if has_bp:
                    bp_sb = persist.tile([1, D], bf16)
                    nc.sync.dma_start(out=bp_sb, in_=bp_d[:])

                # ---- emission helpers ----
                def emit_qk(m):
                    # q,k feature-major: chunk m covers feats 128m..128m+127
                    dst = qT_p if m < 2 else kT_p
                    plane = m % 2
                    for n2 in range(NJ // 2):
                        ps = ps_sc.tile([128, 1024], f32, tag="sc", name="ps")
                        for half in range(2):
                            n = 2 * n2 + half
                            sl = slice(half * 512, (half + 1) * 512)
                            for k in range(DK):
                                nc.tensor.matmul(
                                    ps[:, sl],
                                    wqkvTi_sb[:, k, m * 128:(m + 1) * 128],
                                    xTi_sbs[n][:, k, :],
                                    start=(k == 0),
                                    stop=(k == DK - 1),
                                )
                        if has_bqk:
                            nc.scalar.activation(
                                dst[plane][:, n2 * 1024:(n2 + 1) * 1024],
                                ps,
                                mybir.ActivationFunctionType.Copy,
                                bias=bqk_sb[:, m:m + 1],
                            )
                        else:
                            nc.vector.tensor_copy(
                                out=dst[plane][:, n2 * 1024:(n2 + 1) * 1024], in_=ps
                            )

                def emit_v(lc):
                    # v natural layout [L, feat]
                    psv = ps_sc.tile([128, 1024], f32, tag="sc", name="psv")
                    c, r = divmod(lc, NXB)
                    for k in range(DK):
                        nc.tensor.matmul(
                            psv[:, 0:FV],
                            xTi_sbs[c][:, k, r * 128:(r + 1) * 128],
                            wqkvTi_sb[:, k, FQK:FQKV],
                            start=(k == 0),
                            stop=(k == DK - 1) and not has_bv,
                        )
                    if has_bv:
                        nc.tensor.matmul(
                            psv[:, 0:FV], ones_row[0:1, :], bv_sb,
                            start=False, stop=True,
                        )
                    nc.vector.tensor_copy(
                        out=v_augs[lc][:, :, 0:KHD],
                        in_=psv[:, 0:FV].rearrange("p (h k) -> p h k", h=HPC),
                    )

                def emit_att_pair(j, h0, fillers=()):
                    # two heads (same q/k plane) processed in lockstep, with the
                    # PV matmuls software-pipelined one round behind the scores
                    # so PE never waits on ScalarE's exp.  One filler thunk
                    # (a proj half) is emitted after each round as PE slack.
                    fillers = list(fillers)
                    pl = h0 // 2
                    pos = [(h0 % 2) * 64, ((h0 + 1) % 2) * 64]
                    heads = [h0, h0 + 1]
                    outTs = [
                        ps_out.tile([KHD + 1, 512], f32, tag="outT", name="outT")
                        for _ in range(2)
                    ]
                    qrs = slice(j * 512, (j + 1) * 512)
                    last_c = 4 * j + 3
                    # rounds: below-diagonal pairs then two packed diagonal pairs
                    rounds = [("below", cp) for cp in range(0, 4 * j, 2)]
                    rounds += [("diag", 0), ("diag", 2)]
                    pending = []  # (hh, [(c, colslice in ex)]), ex tile

                    def flush_pending():
                        for hh, parts, ex in pending:
                            for c, exsl, n0 in parts:
                                nc.tensor.matmul(
                                    outTs[hh][:, n0:512],
                                    v_augs[c][:, heads[hh], :],
                                    ex[:, exsl],
                                    start=(c == 0),
                                    stop=(c == last_c),
                                )
                        pending.clear()

                    for kind, arg in rounds:
                        new_pending = []
                        for hh in range(2):
                            po = pos[hh]
                            sc = ps_sc.tile([128, 1024], f32, tag="sc", name="sc")
                            if kind == "below":
                                cp = arg
                                for half in range(2):
                                    c = cp + half
                                    nc.tensor.matmul(
                                        sc[:, half * 512:(half + 1) * 512],
                                        kT_p[pl][po:po + 64, c * 128:(c + 1) * 128],
                                        qT_p[pl][po:po + 64, qrs],
                                        start=True,
                                        stop=True,
                                    )
                                ex = work.tile([128, 1024], bf16, tag="expT", name="ex", bufs=5)
                                nc.scalar.activation(
                                    ex, sc,
                                    mybir.ActivationFunctionType.Exp, scale=SCALE,
                                )
                                parts = [
                                    (cp, slice(0, 512), 0),
                                    (cp + 1, slice(512, 1024), 0),
                                ]
                            else:
                                i0 = arg
                                ws = [512 - 128 * (i0 + di) for di in range(2)]
                                offs = [0, ws[0]]
                                wtot = ws[0] + ws[1]
                                for di in range(2):
                                    c = 4 * j + i0 + di
                                    n0 = 128 * (i0 + di)
                                    nc.tensor.matmul(
                                        sc[:, offs[di]:offs[di] + ws[di]],
                                        kT_p[pl][po:po + 64, c * 128:(c + 1) * 128],
                                        qT_p[pl][po:po + 64, j * 512 + n0:(j + 1) * 512],
                                        start=True,
                                        stop=True,
                                    )
                                ex = work.tile([128, 1024], bf16, tag="expT", name="ex", bufs=5)
                                nc.scalar.activation(
                                    ex[:, 0:wtot], sc[:, 0:wtot],
                                    mybir.ActivationFunctionType.Exp, scale=SCALE,
                                )
                                for di in range(2):
                                    nc.gpsimd.tensor_mul(
                                        ex[:, offs[di]:offs[di] + 128],
                                        ex[:, offs[di]:offs[di] + 128],
                                        tri_sb,
                                    )
                                parts = [
                                    (4 * j + i0, slice(0, ws[0]), 128 * i0),
                                    (
                                        4 * j + i0 + 1,
                                        slice(offs[1], offs[1] + ws[1]),
                                        128 * (i0 + 1),
                                    ),
                                ]
                            new_pending.append((hh, parts, ex))
                        flush_pending()
                        pending.extend(new_pending)
                        if fillers:
                            fillers.pop(0)()
                    flush_pending()
                    for f in fillers:
                        f()
                    # normalize: attnT[f, qrow] = outT[f, qrow] / outT[64, qrow]
                    # last block: split in halves so proj(NJ-1) starts sooner
                    nsplit = 4 if j == NJ - 1 else 1
                    w = 512 // nsplit
                    for s in range(nsplit):
                        csl = slice(s * w, (s + 1) * w)
                        for hh in range(2):
                            po = pos[hh]
                            recip = small.tile([1, w], f32, tag="recip", name="recip")
                            nc.vector.reciprocal(
                                recip, outTs[hh][KHD:KHD + 1, csl]
                            )
                            bc_sb = small.tile([64, w], f32, tag="bcsb", name="bc_sb")
                            nc.gpsimd.partition_broadcast(bc_sb, recip)
                            nc.vector.tensor_mul(
                                attnT_js[j][po:po + 64, pl, csl],
                                outTs[hh][0:KHD, csl],
                                bc_sb,
                            )

                proj_ysb = {}

                def emit_proj_half(j, lq, half):
                    # one 512-col half of one 128-row chunk of proj(j);
                    # small unit usable as PE filler between att rounds
                    lc = 4 * j + lq
                    if half == 0:
                        proj_ysb[j] = work.tile(
                            [128, 1024], bf16, tag="ysb", name="ysb", bufs=3
                        )
                    ysb = proj_ysb[j]
                    sl = slice(half * 512, (half + 1) * 512)
                    psy = ps_y.tile([128, 512], f32, tag="psy", name="psy")
                    for kc in range(2):
                        nc.tensor.matmul(
                            psy,
                            attnT_js[j][:, kc, lq * 128:(lq + 1) * 128],
                            wpTi_sb[:, kc, sl],
                            start=(kc == 0),
                            stop=(kc == 1) and not has_bp,
                        )
                    if has_bp:
                        nc.tensor.matmul(
                            psy, ones_row[0:1, :], bp_sb[0:1, sl],
                            start=False, stop=True,
                        )
                    if j == NJ - 1 and (lq * 2 + half) % 2 == 1:
                        # tail: ACT is free once exp is done; alternate
                        # copies DVE/ACT so proj(3) is PE-paced
                        nc.scalar.activation(
                            ysb[:, sl], psy, mybir.ActivationFunctionType.Copy
                        )
                    else:
                        nc.vector.tensor_copy(out=ysb[:, sl], in_=psy)
                    if j == NJ - 1 and lq == 3:
                        # drain the final block's halves eagerly
                        nc.sync.dma_start(
                            out=y_d[lc * 128:(lc + 1) * 128, sl], in_=ysb[:, sl]
                        )
                    elif half == 1:
                        nc.sync.dma_start(
                            out=y_d[lc * 128:(lc + 1) * 128, :], in_=ysb
                        )

                def proj_fillers(j):
                    return [
                        (lambda j=j, lq=lq, h=h: emit_proj_half(j, lq, h))
                        for lq in range(4)
                        for h in range(2)
                    ]

                # ---- emission order: emit_v first so PE ramps while xT
                # streams in; attention's exp stream starts early so ACT
                # overlaps the QKV phase; proj halves injected as PE filler
                # between att rounds so the tail is only proj(3) ----
                for lc in range(8):
                    emit_v(lc)
                emit_qk(0)
                emit_qk(2)
                emit_att_pair(0, 0, fillers=[
                    (lambda lc=lc: emit_v(lc)) for lc in range(8, 12)
                ])
                emit_qk(1)
                emit_qk(3)
                emit_att_pair(0, 2, fillers=[
                    (lambda lc=lc: emit_v(lc)) for lc in range(12, LC)
                ])
                f0, f1, f2 = proj_fillers(0), proj_fillers(1), proj_fillers(2)
                emit_att_pair(1, 0, fillers=f0[0:3])
                emit_att_pair(1, 2, fillers=f0[3:6])
                emit_att_pair(2, 0, fillers=f0[6:8] + f1[0:2])
                emit_att_pair(2, 2, fillers=f1[2:6])
                emit_att_pair(3, 0, fillers=f1[6:8] + f2[0:3])
                emit_att_pair(3, 2, fillers=f2[3:8])
                for f in proj_fillers(NJ - 1):
                    f()

    nc.compile()
    return nc


def kernel(input_BLD, W_attn, b_attn, W_proj, b_proj):
    from ml_dtypes import bfloat16

    input_BLD = np.asarray(input_BLD, dtype=np.float32)
    W_attn = np.asarray(W_attn, dtype=np.float32)
    b_attn = np.asarray(b_attn, dtype=np.float32)
    W_proj = np.asarray(W_proj, dtype=np.float32)
    b_proj = np.asarray(b_proj, dtype=np.float32)

    has_bqk = bool(np.any(b_attn[: 2 * D]))
    has_bv = bool(np.any(b_attn[2 * D:]))
    has_bp = bool(np.any(b_proj))

    key = (has_bqk, has_bv, has_bp)
    if key not in _CACHE:
        _CACHE[key] = _build(*key)
    nc = _CACHE[key]

    tri = (np.arange(128)[None, :] >= np.arange(128)[:, None]).astype(bfloat16)
    in_maps = []
    for c in range(NCORES):
        b, t = divmod(c, 4)
        hs = t * HPC * KHD  # feature offset of this core's heads
        w_loc = np.concatenate(
            [
                W_attn[hs:hs + FV],  # q rows
                W_attn[D + hs:D + hs + FV],  # k rows
                W_attn[2 * D + hs:2 * D + hs + FV],  # v rows
            ],
            axis=0,
        )  # [768, 1024]
        # k-interleaved host packing: xTi[p, c, k, l] = x[b][c*512+l, k*128+p]
        xT = np.ascontiguousarray(input_BLD[b].T.astype(bfloat16))  # [D, L]
        xTi = np.ascontiguousarray(
            xT.reshape(DK, 128, NXB, 512).transpose(1, 2, 0, 3)
        )  # [128, 4, 8, 512]
        wqkvT = w_loc.T.astype(bfloat16)  # [D, 768]
        wqkvTi = np.ascontiguousarray(
            wqkvT.reshape(DK, 128, FQKV).transpose(1, 0, 2)
        )  # [128, 8, 768]
        wpT = W_proj[:, hs:hs + FV].T.astype(bfloat16)  # [256, 1024]
        wpTi = np.ascontiguousarray(
            wpT.reshape(2, 128, D).transpose(1, 0, 2)
        )  # [128, 2, 1024]
        m = {
            "xTi": xTi,
            "wqkvTi": wqkvTi,
            "wpTi": wpTi,
            "tri": tri,
        }
        if has_bqk:
            bqk = np.concatenate([b_attn[hs:hs + FV], b_attn[D + hs:D + hs + FV]])
            m["bqk"] = np.ascontiguousarray(bqk.reshape(FQK // 128, 128).T)
        if has_bv or has_bp:
            m["onesrow"] = np.ones((1, 128), bfloat16)
        if has_bv:
            m["bv"] = b_attn[2 * D + hs:2 * D + hs + FV][None, :].astype(bfloat16)
        if has_bp:
            m["bp"] = (b_proj / 4.0)[None, :].astype(bfloat16)
        in_maps.append(m)

    from concourse.bass_utils import run_bass_kernel_spmd

    globals()["_last_in_maps"] = in_maps
    res = run_bass_kernel_spmd(nc, in_maps, list(range(NCORES)))
    globals()["_LAST_RESULTS"] = res
    out = np.empty((B, L, D), dtype=np.float32)
    for b in range(B):
        acc = res.results[4 * b]["y"].astype(np.float32)
        for t in range(1, 4):
            acc = acc + res.results[4 * b + t]["y"].astype(np.float32)
        out[b] = acc
    return out


# revision 25
# speedup vs baseline: 44.4599x; 44.4599x over previous
"""Causal self-attention Trainium2 Bass kernel (v2, bf16 pipeline).

Problem: nn_CausalSelfAttention (B=2, L=2048, D=1024, H=16 heads, Khd=64).

Sharding (8 cores): data-parallel over B (2 way) x tensor-parallel over
heads (4 way, 4 heads/core).  Each core computes
  qkv_local = x_b @ W_attn_local.T          (c_attn column-sharded)
  attn_local = causal_attention(q,k,v)      (4 heads)
  y_partial  = attn_local @ W_proj_local.T  (c_proj row-sharded)
and the host sums the 4 partials per batch (the row-parallel unshard).

v2 changes over the fp32r baseline:
  - whole pipeline in bf16 (fp32 PSUM accumulation): halves DMA bytes,
    removes the fp32r 4x penalty on <256-wide matmuls, halves SBUF.
  - inputs host-packed into k-interleaved layouts so each big tensor
    loads with ONE wide DMA (9 input DMAs total vs ~60): kills the
    ~630ns/DMA HWDGE serialization that stalled the start.
  - v_aug ones-column via memset instead of 16 tiny DMAs.
  - emission starts with emit_v on the first xT column block so PE
    ramps while the rest of xT streams in.
  - q/k PSUM->SBUF copies split DVE/ACT; tail proj copies on ACT.
  - y written bf16 via [128,1024] staging tiles (16 output DMAs).
"""

import math

import numpy as np

B, L, D, H = 2, 2048, 1024, 16
KHD = D // H  # 64 head dim
NCORES = 8
HPC = 4  # heads per core
FQK = 2 * HPC * KHD  # 512 q+k local features
FV = HPC * KHD  # 256 v local features
FQKV = FQK + FV  # 768
DK = D // 128  # 8 contraction chunks
LC = L // 128  # 16 row chunks
NJ = L // 512  # 4 qrow blocks
NXB = 4  # xT column blocks of 512
SCALE = 1.0 / math.sqrt(KHD)

_CACHE = {}


def _build(has_bqk: bool, has_bv: bool, has_bp: bool, reps: int = 1):
    import concourse.bass as bass
    import concourse.mybir as mybir
    import concourse.tile as tile
    from concourse import bacc

    f32 = mybir.dt.float32
    bf16 = mybir.dt.bfloat16

    nc = bacc.Bacc(None, target_bir_lowering=False)
    # host-packed k-interleaved layouts: one wide DMA per tensor
    xTi_d = nc.declare_dram_parameter("xTi", [128, NXB, DK, 512], bf16, isOutput=False)
    wqkvTi_d = nc.declare_dram_parameter("wqkvTi", [128, DK, FQKV], bf16, isOutput=False)
    wpTi_d = nc.declare_dram_parameter("wpTi", [128, 2, D], bf16, isOutput=False)
    tri_d = nc.declare_dram_parameter("tri", [128, 128], bf16, isOutput=False)
    if has_bqk:
        bqk_d = nc.declare_dram_parameter("bqk", [128, FQK // 128], f32, isOutput=False)
    if has_bv or has_bp:
        onesrow_d = nc.declare_dram_parameter("onesrow", [1, 128], bf16, isOutput=False)
    if has_bv:
        bv_d = nc.declare_dram_parameter("bv", [1, FV], bf16, isOutput=False)
    if has_bp:
        bp_d = nc.declare_dram_parameter("bp", [1, D], bf16, isOutput=False)
    y_d = nc.declare_dram_parameter("y", [L, D], bf16, isOutput=True)

    with nc.allow_low_precision(reason="bf16 matmul pipeline, fp32 accum"), tile.TileContext(nc) as tc:
        with (
            tc.tile_pool(name="persist", bufs=1) as persist,
            tc.tile_pool(name="work", bufs=3) as work,
            tc.tile_pool(name="small", bufs=2) as small,
            tc.tile_pool(name="ps_sc", bufs=2, space="PSUM") as ps_sc,
            tc.tile_pool(name="ps_out", bufs=2, space="PSUM") as ps_out,
            tc.tile_pool(name="ps_y", bufs=2, space="PSUM") as ps_y,
        ):
            for _rep in range(reps):
                # ---- persistent SBUF tensors ----
                xTi_sbs = [
                    persist.tile([128, DK, 512], bf16, name=f"xTi{c}", tag=f"xTi{c}")
                    for c in range(NXB)
                ]
                wqkvTi_sb = persist.tile([128, DK, FQKV], bf16, tag="wqkvTi")
                qT_p = [persist.tile([128, L], bf16, name=f"qT{p}", tag=f"qT{p}") for p in range(2)]
                kT_p = [persist.tile([128, L], bf16, name=f"kT{p}", tag=f"kT{p}") for p in range(2)]
                v_augs = [
                    persist.tile([128, HPC, KHD + 1], bf16, name=f"vaug{lc}", tag=f"vaug{lc}")
                    for lc in range(LC)
                ]
                attnT_js = [
                    persist.tile([128, 2, 512], bf16, name=f"attnT{j}", tag=f"attnT{j}")
                    for j in range(NJ)
                ]
                wpTi_sb = persist.tile([128, 2, D], bf16, tag="wpTi")
                tri_sb = persist.tile([128, 128], bf16)

                # ---- input DMAs: few, wide, priority-ordered; none on ACT
                # (it needs every cycle for exp).  First emit_v(0..3) needs
                # only xTi[0] cols 0:128-per-k and wqkvTi v-cols: load those
                # first so PE starts ~3us in.
                nc.sync.dma_start(
                    out=xTi_sbs[0][:, :, 0:128], in_=xTi_d[:, 0, :, 0:128]
                )
                nc.gpsimd.dma_start(
                    out=wqkvTi_sb[:, :, FQK:FQKV], in_=wqkvTi_d[:, :, FQK:FQKV]
                )
                nc.sync.dma_start(
                    out=xTi_sbs[0][:, :, 128:512], in_=xTi_d[:, 0, :, 128:512]
                )
                nc.gpsimd.dma_start(
                    out=wqkvTi_sb[:, :, 0:FQK], in_=wqkvTi_d[:, :, 0:FQK]
                )
                for c in range(1, NXB):
                    nc.sync.dma_start(out=xTi_sbs[c], in_=xTi_d[:, c])
                nc.gpsimd.dma_start(out=wpTi_sb, in_=wpTi_d[:])
                nc.gpsimd.dma_start(out=tri_sb, in_=tri_d[:])
                # ones column of v_aug (softmax denominator trick)
                for lc in range(LC):
                    nc.vector.memset(v_augs[lc][:, :, KHD], 1.0)
                if has_bqk:
                    bqk_sb = persist.tile([128, FQK // 128], f32)
                    nc.sync.dma_start(out=bqk_sb, in_=bqk_d[:])
                if has_bv or has_bp:
                    ones_row = persist.tile([1, 128], bf16)
                    nc.sync.dma_start(out=ones_row, in_=onesrow_d[:])
                if has_bv:
                    bv_sb = persist.tile([1, FV], bf16)
                    nc.sync.dma_start(out=bv_sb, in_=bv_d[:])
                if has_bp:
                    bp_sb = persist.tile([1, D], bf16)
                    nc.sync.dma_start(out=bp_sb, in_=bp_d[:])

                # ---- emission helpers ----
                def emit_qk(m):
                    # q,k feature-major: chunk m covers feats 128m..128m+127
                    dst = qT_p if m < 2 else kT_p
                    plane = m % 2
                    for n2 in range(NJ // 2):
                        ps = ps_sc.tile([128, 1024], f32, tag="sc", name="ps")
                        for half in range(2):
                            n = 2 * n2 + half
                            sl = slice(half * 512, (half + 1) * 512)
                            for k in range(DK):
                                nc.tensor.matmul(
                                    ps[:, sl],
                                    wqkvTi_sb[:, k, m * 128:(m + 1) * 128],
                                    xTi_sbs[n][:, k, :],
                                    start=(k == 0),
                                    stop=(k == DK - 1),
                                )
                        if has_bqk:
                            nc.scalar.activation(
                                dst[plane][:, n2 * 1024:(n2 + 1) * 1024],
                                ps,
                                mybir.ActivationFunctionType.Copy,
                                bias=bqk_sb[:, m:m + 1],
                            )
                        else:
                            nc.vector.tensor_copy(
                                out=dst[plane][:, n2 * 1024:(n2 + 1) * 1024], in_=ps
                            )

                def emit_v(lc):
                    # v natural layout [L, feat]
                    psv = ps_sc.tile([128, 1024], f32, tag="sc", name="psv")
                    c, r = divmod(lc, NXB)
                    for k in range(DK):
                        nc.tensor.matmul(
                            psv[:, 0:FV],
                            xTi_sbs[c][:, k, r * 128:(r + 1) * 128],
                            wqkvTi_sb[:, k, FQK:FQKV],
                            start=(k == 0),
                            stop=(k == DK - 1) and not has_bv,
                        )
                    if has_bv:
                        nc.tensor.matmul(
                            psv[:, 0:FV], ones_row[0:1, :], bv_sb,
                            start=False, stop=True,
                        )
                    nc.vector.tensor_copy(
                        out=v_augs[lc][:, :, 0:KHD],
                        in_=psv[:, 0:FV].rearrange("p (h k) -> p h k", h=HPC),
                    )

                def emit_att_pair(j, h0, fillers=()):
                    # two heads (same q/k plane) processed in lockstep, with the
                    # PV matmuls software-pipelined one round behind the scores
                    # so PE never waits on ScalarE's exp.  One filler thunk
                    # (a proj half) is emitted after each round as PE slack.
                    fillers = list(fillers)
                    pl = h0 // 2
                    pos = [(h0 % 2) * 64, ((h0 + 1) % 2) * 64]
                    heads = [h0, h0 + 1]
                    outTs = [
                        ps_out.tile([KHD + 1, 512], f32, tag="outT", name="outT")
                        for _ in range(2)
                    ]
                    qrs = slice(j * 512, (j + 1) * 512)
                    last_c = 4 * j + 3
                    # rounds: below-diagonal pairs then two packed diagonal pairs
                    rounds = [("below", cp) for cp in range(0, 4 * j, 2)]
                    rounds += [("diag", 0), ("diag", 2)]
                    pending = []  # (hh, [(c, colslice in ex)]), ex tile

                    def flush_pending():
                        for hh, parts, ex in pending:
                            for c, exsl, n0 in parts:
                                nc.tensor.matmul(
                                    outTs[hh][:, n0:512],
                                    v_augs[c][:, heads[hh], :],
                                    ex[:, exsl],
                                    start=(c == 0),
                                    stop=(c == last_c),
                                )
                        pending.clear()

                    for kind, arg in rounds:
                        new_pending = []
                        for hh in range(2):
                            po = pos[hh]
                            sc = ps_sc.tile([128, 1024], f32, tag="sc", name="sc")
                            if kind == "below":
                                cp = arg
                                for half in range(2):
                                    c = cp + half
                                    nc.tensor.matmul(
                                        sc[:, half * 512:(half + 1) * 512],
                                        kT_p[pl][po:po + 64, c * 128:(c + 1) * 128],
                                        qT_p[pl][po:po + 64, qrs],
                                        start=True,
                                        stop=True,
                                    )
                                ex = work.tile([128, 1024], bf16, tag="expT", name="ex", bufs=5)
                                nc.scalar.activation(
                                    ex, sc,
                                    mybir.ActivationFunctionType.Exp, scale=SCALE,
                                )
                                parts = [
                                    (cp, slice(0, 512), 0),
                                    (cp + 1, slice(512, 1024), 0),
                                ]
                            else:
                                i0 = arg
                                ws = [512 - 128 * (i0 + di) for di in range(2)]
                                offs = [0, ws[0]]
                                wtot = ws[0] + ws[1]
                                for di in range(2):
                                    c = 4 * j + i0 + di
                                    n0 = 128 * (i0 + di)
                                    nc.tensor.matmul(
                                        sc[:, offs[di]:offs[di] + ws[di]],
                                        kT_p[pl][po:po + 64, c * 128:(c + 1) * 128],
                                        qT_p[pl][po:po + 64, j * 512 + n0:(j + 1) * 512],
                                        start=True,
                                        stop=True,
                                    )
                                ex = work.tile([128, 1024], bf16, tag="expT", name="ex", bufs=5)
                                nc.scalar.activation(
                                    ex[:, 0:wtot], sc[:, 0:wtot],
                                    mybir.ActivationFunctionType.Exp, scale=SCALE,
                                )
                                for di in range(2):
                                    nc.vector.tensor_mul(
                                        ex[:, offs[di]:offs[di] + 128],
                                        ex[:, offs[di]:offs[di] + 128],
                                        tri_sb,
                                    )
                                parts = [
                                    (4 * j + i0, slice(0, ws[0]), 128 * i0),
                                    (
                                        4 * j + i0 + 1,
                                        slice(offs[1], offs[1] + ws[1]),
                                        128 * (i0 + 1),
                                    ),
                                ]
                            new_pending.append((hh, parts, ex))
                        flush_pending()
                        pending.extend(new_pending)
                        if fillers:
                            fillers.pop(0)()
                    flush_pending()
                    for f in fillers:
                        f()
                    # normalize: attnT[f, qrow] = outT[f, qrow] / outT[64, qrow]
                    # last block: split in halves so proj(NJ-1) starts sooner
                    nsplit = 4 if j == NJ - 1 else 1
                    w = 512 // nsplit
                    for s in range(nsplit):
                        csl = slice(s * w, (s + 1) * w)
                        for hh in range(2):
                            po = pos[hh]
                            recip = small.tile([1, w], f32, tag="recip", name="recip")
                            nc.vector.reciprocal(
                                recip, outTs[hh][KHD:KHD + 1, csl]
                            )
                            bc_sb = small.tile([64, w], f32, tag="bcsb", name="bc_sb")
                            nc.gpsimd.partition_broadcast(bc_sb, recip)
                            nc.vector.tensor_mul(
                                attnT_js[j][po:po + 64, pl, csl],
                                outTs[hh][0:KHD, csl],
                                bc_sb,
                            )

                proj_ysb = {}

                def emit_proj_half(j, lq, half):
                    # one 512-col half of one 128-row chunk of proj(j);
                    # small unit usable as PE filler between att rounds
                    lc = 4 * j + lq
                    if half == 0:
                        proj_ysb[j] = work.tile(
                            [128, 1024], bf16, tag="ysb", name="ysb", bufs=3
                        )
                    ysb = proj_ysb[j]
                    sl = slice(half * 512, (half + 1) * 512)
                    psy = ps_y.tile([128, 512], f32, tag="psy", name="psy")
                    for kc in range(2):
                        nc.tensor.matmul(
                            psy,
                            attnT_js[j][:, kc, lq * 128:(lq + 1) * 128],
                            wpTi_sb[:, kc, sl],
                            start=(kc == 0),
                            stop=(kc == 1) and not has_bp,
                        )
                    if has_bp:
                        nc.tensor.matmul(
                            psy, ones_row[0:1, :], bp_sb[0:1, sl],
                            start=False, stop=True,
                        )
                    if j == NJ - 1 and (lq * 2 + half) % 2 == 1:
                        # tail: ACT is free once exp is done; alternate
                        # copies DVE/ACT so proj(3) is PE-paced
                        nc.scalar.activation(
                            ysb[:, sl], psy, mybir.ActivationFunctionType.Copy
                        )
                    else:
                        nc.vector.tensor_copy(out=ysb[:, sl], in_=psy)
                    if j == NJ - 1 and lq == 3:
                        # drain the final block's halves eagerly
                        nc.sync.dma_start(
                            out=y_d[lc * 128:(lc + 1) * 128, sl], in_=ysb[:, sl]
                        )
                    elif half == 1:
                        nc.sync.dma_start(
                            out=y_d[lc * 128:(lc + 1) * 128, :], in_=ysb
                        )

                def proj_fillers(j):
                    return [
                        (lambda j=j, lq=lq, h=h: emit_proj_half(j, lq, h))
                        for lq in range(4)
                        for h in range(2)
                    ]

                # ---- emission order: emit_v first so PE ramps while xT
                # streams in; attention's exp stream starts early so ACT
                # overlaps the QKV phase; proj halves injected as PE filler
                # between att rounds so the tail is only proj(3) ----
                for lc in range(8):
                    emit_v(lc)
                emit_qk(0)
                emit_qk(2)
                emit_att_pair(0, 0, fillers=[
                    (lambda lc=lc: emit_v(lc)) for lc in range(8, 12)
                ])
                emit_qk(1)
                emit_qk(3)
                emit_att_pair(0, 2, fillers=[
                    (lambda lc=lc: emit_v(lc)) for lc in range(12, LC)
                ])
                f0, f1, f2 = proj_fillers(0), proj_fillers(1), proj_fillers(2)
                emit_att_pair(1, 0, fillers=f0[0:3])
                emit_att_pair(1, 2, fillers=f0[3:6])
                emit_att_pair(2, 0, fillers=f0[6:8] + f1[0:2])
                emit_att_pair(2, 2, fillers=f1[2:6])
                emit_att_pair(3, 0, fillers=f1[6:8] + f2[0:3])
                emit_att_pair(3, 2, fillers=f2[3:8])
                for f in proj_fillers(NJ - 1):
                    f()

    nc.compile()
    return nc


def kernel(input_BLD, W_attn, b_attn, W_proj, b_proj):
    from ml_dtypes import bfloat16

    input_BLD = np.asarray(input_BLD, dtype=np.float32)
    W_attn = np.asarray(W_attn, dtype=np.float32)
    b_attn = np.asarray(b_attn, dtype=np.float32)
    W_proj = np.asarray(W_proj, dtype=np.float32)
    b_proj = np.asarray(b_proj, dtype=np.float32)

    has_bqk = bool(np.any(b_attn[: 2 * D]))
    has_bv = bool(np.any(b_attn[2 * D:]))
    has_bp = bool(np.any(b_proj))

    key = (has_bqk, has_bv, has_bp)
    if key not in _CACHE:
        _CACHE[key] = _build(*key)
    nc = _CACHE[key]

    tri = (np.arange(128)[None, :] >= np.arange(128)[:, None]).astype(bfloat16)
    in_maps = []
    for c in range(NCORES):
        b, t = divmod(c, 4)
        hs = t * HPC * KHD  # feature offset of this core's heads
        w_loc = np.concatenate(
            [
                W_attn[hs:hs + FV],  # q rows
                W_attn[D + hs:D + hs + FV],  # k rows
                W_attn[2 * D + hs:2 * D + hs + FV],  # v rows
            ],
            axis=0,
        )  # [768, 1024]
        # k-interleaved host packing: xTi[p, c, k, l] = x[b][c*512+l, k*128+p]
        xT = np.ascontiguousarray(input_BLD[b].T.astype(bfloat16))  # [D, L]
        xTi = np.ascontiguousarray(
            xT.reshape(DK, 128, NXB, 512).transpose(1, 2, 0, 3)
        )  # [128, 4, 8, 512]
        wqkvT = w_loc.T.astype(bfloat16)  # [D, 768]
        wqkvTi = np.ascontiguousarray(
            wqkvT.reshape(DK, 128, FQKV).transpose(1, 0, 2)
        )  # [128, 8, 768]
        wpT = W_proj[:, hs:hs + FV].T.astype(bfloat16)  # [256, 1024]
        wpTi = np.ascontiguousarray(
            wpT.reshape(2, 128, D).transpose(1, 0, 2)
        )  # [128, 2, 1024]
        m = {
            "xTi": xTi,
            "wqkvTi": wqkvTi,
            "wpTi": wpTi,
            "tri": tri,
        }
        if has_bqk:
            bqk = np.concatenate([b_attn[hs:hs + FV], b_attn[D + hs:D + hs + FV]])
            m["bqk"] = np.ascontiguousarray(bqk.reshape(FQK // 128, 128).T)
        if has_bv or has_bp:
            m["onesrow"] = np.ones((1, 128), bfloat16)
        if has_bv:
            m["bv"] = b_attn[2 * D + hs:2 * D + hs + FV][None, :].astype(bfloat16)
        if has_bp:
            m["bp"] = (b_proj / 4.0)[None, :].astype(bfloat16)
        in_maps.append(m)

    from concourse.bass_utils import run_bass_kernel_spmd

    globals()["_last_in_maps"] = in_maps
    res = run_bass_kernel_spmd(nc, in_maps, list(range(NCORES)))
    globals()["_LAST_RESULTS"] = res
    out = np.empty((B, L, D), dtype=np.float32)
    for b in range(B):
        acc = res.results[4 * b]["y"].astype(np.float32)
        for t in range(1, 4):
            acc = acc + res.results[4 * b + t]["y"].astype(np.float32)
        out[b] = acc
    return out
